# revision 13
# baseline (speedup 1.0000x reference)
"""Trainium2 Bass kernel for per-node temporal graph conv (LCN).

Math (matches the reference): for each node v with neighbor list idx[v]
(chain graph: v-1, v, v+1, masked at the ends),
    out[n,o,v,t] = b[v,o] + sum_{k,c,kt} x_pad[n,c,idx[v,k],t+kt] * Wm[v,o,c,k,kt]

Strategy: data-parallel over batch N across 8 cores (2 samples each);
weights/bias replicated. The host pre-packs x (bf16) into the exact
SBUF layout of 13 "node pair" blocks of 514 columns (512 + 2 temporal
zero pads):
    partitions  0- 63: node 2j-1 (odd nodes; block 0 holds a zero ghost)
    partitions 64-127: node 2j   (even nodes)
so one [128, 512] slice at column offset j*514+kt stacks two adjacent
nodes' time-shifted frames on the contraction dim, and every device DMA
is a single fully contiguous line-rate transfer. Outputs are computed
per node pair (v=2m, 2m+1) stacked on the PSUM partition dim (M=128):
6 accumulating bf16 matmuls (3 temporal taps x 2 source blocks) per
pair, with weight blocks pre-scattered host-side so every (v,u) tap
lands in exactly one matmul. Bias is fused into the PSUM->SBUF copy,
which writes fp16 into per-chunk staging tiles that stream out as
contiguous DMAs; the host casts back to fp32 and unpacks.

Pipeline shaping: weight loads go on the scalar HWDGE ring while x
loads go on the sync ring so they stream in parallel; the first chunks
are small so the first real matmul starts as soon as possible; junk
matmuls at kernel start keep the PE busy through the DMA fill so the
HAM clock gate is warm (2.4 GHz) when real matmuls begin.
"""

import numpy as np
import ml_dtypes

import concourse.bacc as bacc
import concourse.mybir as mybir
from concourse.tile import TileContext
from concourse.bass_utils import run_bass_kernel_spmd

V, K, CIN, COUT, N, T, TK = 25, 3, 64, 64, 16, 512, 3
NCORES = 8
NPER = N // NCORES          # samples per core
TP = T + 2                  # block width incl. temporal zero pads
NB = (V + 1) // 2           # node-pair blocks
NSLOT = 6 * (NB - 1) + TK   # distinct [128,128] weight tiles (75)
WCH = [(0, 6), (6, 30), (30, 54), (54, 75)]   # weight-load slot chunks
XCH = [(0, 3), (3, 7), (7, 10), (10, NB)]     # x block chunks (per sample)
YCH = [(0, 7), (7, 11), (11, 12), (12, NB)]   # y block chunks (per sample)
NWARM = 26                  # junk matmuls to pre-warm the PE clock gate

_BF16 = mybir.dt.bfloat16
_F16 = mybir.dt.float16
_F32 = mybir.dt.float32

_cache = {}


def _slot(m, kt, mmi):
    # block-major slot numbering so early blocks' weights arrive first
    return m * 6 + kt * 2 + mmi if m < NB - 1 else 6 * (NB - 1) + kt


def _chunk(ranges, b):
    for ci, (b0, b1) in enumerate(ranges):
        if b0 <= b < b1:
            return ci, b0
    raise AssertionError


def _build_program():
    nc = bacc.Bacc("TRN2", num_devices=NCORES)
    x_in = nc.dram_tensor("x", [NPER, 128, NB * TP], _BF16, kind="ExternalInput")
    wl_in = nc.dram_tensor("wl", [128, NSLOT * 128], _BF16, kind="ExternalInput")
    b_in = nc.dram_tensor("bias", [128, NB], _F32, kind="ExternalInput")
    y_out = nc.dram_tensor("y", [NPER, 128, NB * T], _F16, kind="ExternalOutput")

    with TileContext(nc) as tc:
        with (
            tc.tile_pool(name="w", bufs=1) as wp,
            tc.tile_pool(name="x", bufs=1) as xp,
            tc.tile_pool(name="ps", bufs=7, space="PSUM") as pp,
            tc.tile_pool(name="o", bufs=1) as op,
        ):
            # PE warm-up: junk matmuls on a zeroed tile into a scratch
            # PSUM bank, running while the DMAs fill SBUF. No data deps.
            wj = wp.tile([128, 512], _BF16, tag="warm", name="wj")
            nc.gpsimd.memset(wj[:, 0:256], 0.0)
            wps = pp.tile([128, 512], _F32, tag="wps", bufs=1, name="wps")
            for _ in range(NWARM):
                nc.tensor.matmul(
                    wps[:, 0:128], lhsT=wj[:, 0:128], rhs=wj[:, 0:128],
                    start=True, stop=True,
                )

            # All loads on the sync HWDGE ring (FIFO) in exact
            # consumption order: weights for blocks [b0,b1) then both
            # samples' x for the same range, repeating.
            wls = [None] * len(WCH)
            xs = {}
            b_sb = None

            def load_w(i):
                s0, s1 = WCH[i]
                t = wp.tile(
                    [128, (s1 - s0) * 128], _BF16, tag=f"wl{i}", name=f"wl{i}"
                )
                nc.sync.dma_start(out=t[:, :], in_=wl_in[:, s0 * 128 : s1 * 128])
                wls[i] = t

            def load_x(ci):
                b0, b1 = XCH[ci]
                for n in range(NPER):
                    t = xp.tile(
                        [128, (b1 - b0) * TP], _BF16,
                        tag=f"xs{n}_{ci}", name=f"xs{n}_{ci}",
                    )
                    nc.sync.dma_start(out=t[:, :], in_=x_in[n, :, b0 * TP : b1 * TP])
                    xs[(n, ci)] = t

            load_w(0)
            b_sb = wp.tile([128, NB], _F32, tag="bias", name="bias")
            nc.sync.dma_start(out=b_sb[:, :], in_=b_in[:, :])
            load_x(0)
            load_w(1)
            load_x(1)
            load_w(2)
            load_x(2)
            load_w(3)
            load_x(3)

            ys = {}
            for n in range(NPER):
                for ci, (b0, b1) in enumerate(YCH):
                    ys[(n, ci)] = op.tile(
                        [128, (b1 - b0) * T], _F16,
                        tag=f"y{n}_{ci}", name=f"y{n}_{ci}",
                    )

            for m in range(NB):
                taps = [
                    (kt, mmi)
                    for kt in range(TK)
                    for mmi in range(2)
                    if 2 * m + mmi < V
                ]
                for n in range(NPER):
                    ps = pp.tile([128, T], _F32, name="ps")
                    for i, (kt, mmi) in enumerate(taps):
                        slot = _slot(m, kt, mmi)
                        sb = m + mmi  # source block
                        ci, cb0 = _chunk(XCH, sb)
                        col = (sb - cb0) * TP + kt
                        wi, ws0 = _chunk(WCH, slot)
                        wc = (slot - ws0) * 128
                        nc.tensor.matmul(
                            ps[:, :],
                            lhsT=wls[wi][:, wc : wc + 128],
                            rhs=xs[(n, ci)][:, col : col + T],
                            start=(i == 0),
                            stop=(i == len(taps) - 1),
                        )
                    ci, cb0 = _chunk(YCH, m)
                    mo = m - cb0
                    nc.vector.tensor_scalar_add(
                        out=ys[(n, ci)][:, mo * T : (mo + 1) * T],
                        in0=ps[:, :],
                        scalar1=b_sb[:, m : m + 1],
                    )
            for ci, (b0, b1) in enumerate(YCH):
                for n in range(NPER):
                    nc.sync.dma_start(
                        out=y_out[n, :, b0 * T : b1 * T], in_=ys[(n, ci)][:, :]
                    )

    nc.compile()
    return nc


def _prep_weights(W, b, idx, mask):
    W = np.asarray(W, np.float32)
    b = np.asarray(b, np.float32)
    idx = np.asarray(idx)
    mask = np.asarray(mask)
    Wm = np.where(mask[:, None, None, :, None], W, 0.0)  # [V,O,C,K,TK]
    W4 = np.zeros((V, V, COUT, CIN, TK), np.float32)
    for v in range(V):
        for k in range(K):
            if mask[v, k]:
                W4[v, idx[v, k]] = Wm[v, :, :, k, :]
    wl = np.zeros((128, NSLOT * 128), np.float32)
    for m in range(NB):
        for kt in range(TK):
            for mmi in range(2):
                if 2 * m + mmi >= V:
                    continue
                slot = _slot(m, kt, mmi)
                blk = m + mmi
                for uh, u in ((0, 2 * blk - 1), (1, 2 * blk)):
                    for vloc in range(2):
                        v = 2 * m + vloc
                        if 0 <= u < V and v < V:
                            # lhsT[64*uh + c, 64*vloc + o] = W4[v,u,o,c,kt]
                            wl[
                                64 * uh : 64 * uh + 64,
                                slot * 128 + 64 * vloc : slot * 128 + 64 * vloc + 64,
                            ] = W4[v, u, :, :, kt].T
    bias = np.zeros((128, NB), np.float32)
    for m in range(NB):
        for vloc in range(2):
            if 2 * m + vloc < V:
                bias[64 * vloc : 64 * vloc + 64, m] = b[2 * m + vloc]
    return wl.astype(ml_dtypes.bfloat16), bias


def _pack_x(x):
    """[N, C, V, T] fp32 -> [N, 128, NB*TP] bf16 in the SBUF pair-block layout."""
    x16 = np.asarray(x, np.float32).astype(ml_dtypes.bfloat16)
    xs = np.zeros((N, 128, NB, TP), ml_dtypes.bfloat16)
    xs[:, 64:, :, 1 : T + 1] = x16[:, :, 0::2, :]
    xs[:, :64, 1:, 1 : T + 1] = x16[:, :, 1::2, :]
    return np.ascontiguousarray(xs.reshape(N, 128, NB * TP))


def _unpack_y(y_dev):
    """[N, 128, NB*T] fp16 -> [N, O, V, T] fp32."""
    y = np.asarray(y_dev).reshape(N, 2, COUT, NB, T).astype(np.float32)
    y = y.transpose(0, 2, 3, 1, 4).reshape(N, COUT, 2 * NB, T)
    return np.ascontiguousarray(y[:, :, :V, :])


def kernel(x, W, b, idx, mask):
    if "nc" not in _cache:
        _cache["nc"] = _build_program()
    nc = _cache["nc"]
    wl, bias = _prep_weights(W, b, idx, mask)
    xp = _pack_x(x)
    in_maps = [
        {"x": xp[c * NPER : (c + 1) * NPER], "wl": wl, "bias": bias}
        for c in range(NCORES)
    ]
    res = run_bass_kernel_spmd(nc, in_maps, list(range(NCORES)))
    y_dev = np.concatenate([res.results[c]["y"] for c in range(NCORES)], axis=0)
    return _unpack_y(y_dev)
